# revision 1
# baseline (speedup 1.0000x reference)
"""Trainium2 Bass kernel for the random-fern VQ-codebook problem (nn_CTE_37512244364031).

Strategy: data-parallel over batch N across 8 NeuronCores (8 images/core).
Per-core fused pipeline, all in on-chip SBUF/PSUM:

  stage A (per fern m, per bit k), layout [x=120 partitions, (n, y)]:
    - diff z = x[c1] shifted - x[c2] shifted, via two PE matmuls with +/-
      shifted-identity weights (partition shift encodes dx), PSUM fp32.
    - h = tanh(5*z - 5*thr)  on ACT (PSUM -> SBUF fp16).
      bit_k = (h > 0) matches sigmoid((d-thr)/T) > 0.5;
      conf factor max(s, 1-s) = (1 + |h|)/2   (2^-12 folded into constants).
    - word = sum_k 2^k bit_k via PE identity-matmul PSUM accumulation of
      tensor_scalar bits; drained to int32 (gather offsets).
    - conf = prod_k (1 + |h_k|) via DVE tensor_scalar(abs_max,add) + mult tree.
  stage B (per image):
    - per fern: SWDGE indirect-DMA gather of 64B fp16 rows from the HBM
      table (table pre-scaled by 2^-6, fp16), conf-weighted on DVE
      (pair-duplicated conf for 2x packing), fern-accumulated via PE
      identity matmuls into PSUM.
    - pool over y via DVE masked cumsum (tensor_tensor_scan) + shifted
      subtract; pool over x + transpose via PE matmul against a banded
      ones matrix (scaled by 2^-6/49); PSUM [y', (x')] DMA'd straight out.

kernel(**inputs) takes the FULL inputs and returns the FULL (64, 415872) output.
"""

import os
import sys
from contextlib import ExitStack

import numpy as np

sys.path.insert(0, "/opt/trn_rl_repo")

M, K, L, D = 8, 12, 9, 32
N, C, H, W = 64, 8, 128, 128
HP = WP = 120
PO = 114
TEMP = 0.1
NCORES = 8
NLOC = N // NCORES          # 8 images per core
NG = 2                      # images per pipeline group
GROUPS = NLOC // NG
YCH = 8                     # y-chunks for fern-accum matmuls (15 y's each)
TSCALE = 2.0 ** -6          # folded into table
BXSCALE = 2.0 ** -6 / 49.0  # folded into pooling matrix

_CACHE = {}


def _host_constants(table):
    tbl16 = np.ascontiguousarray((table.reshape(M * 4096, D) * TSCALE).astype(np.float16))

    bx = np.zeros((HP, PO), np.float32)
    for xp in range(PO):
        bx[xp : xp + 7, xp] = BXSCALE
    bx16 = bx  # fp32

    # shifted +/- identities for the diff matmuls: ipm[p, s, j] = +1 if p==j+s
    # (s in 0..8), ipm[p, 9+s, j] = -1 if p==j+s.
    ipm = np.zeros((128, 18, HP), np.float32)
    for s in range(9):
        for j in range(HP):
            ipm[j + s, s, j] = 1.0
            ipm[j + s, 9 + s, j] = -1.0

    i120 = np.eye(HP, dtype=np.float16)

    return tbl16, bx16, ipm, i120


def _host_biases(thresholds):
    # per-(m,k) threshold negatives (exact, unscaled) + a zeros column,
    # replicated across all 128 partitions: [128, M*K + 1]
    b = np.zeros((1, M * K + 1), np.float32)
    b[0, : M * K] = -thresholds.reshape(M * K)
    return np.ascontiguousarray(np.repeat(b, 128, axis=0))


def _build(ctx, nc, tc, thresholds, chan_idx, offsets):
    import concourse.bass as bass
    from concourse import mybir

    f32 = mybir.dt.float32
    f16 = mybir.dt.float16
    i32 = mybir.dt.int32
    Alu = mybir.AluOpType
    Act = mybir.ActivationFunctionType

    xt_d = nc.dram_tensor("xt", [128, NLOC, C, H], f32, kind="ExternalInput").ap()
    tbl_d = nc.dram_tensor("tbl", [M * 4096, D], f16, kind="ExternalInput").ap()
    bx_d = nc.dram_tensor("bx", [HP, PO], f32, kind="ExternalInput").ap()
    ipm_d = nc.dram_tensor("ipm", [128, 18, HP], f32, kind="ExternalInput").ap()
    i120_d = nc.dram_tensor("i120", [HP, HP], f16, kind="ExternalInput").ap()
    thrb_d = nc.dram_tensor("thrb", [128, M * K + 1], f32, kind="ExternalInput").ap()
    out_d = nc.dram_tensor("out", [NLOC, D, PO, PO], f32, kind="ExternalOutput").ap()

    const = ctx.enter_context(tc.tile_pool(name="const", bufs=1))
    zp_pool = ctx.enter_context(tc.tile_pool(name="zp", bufs=2, space="PSUM"))
    wp_pool = ctx.enter_context(tc.tile_pool(name="wp", bufs=2, space="PSUM"))
    vp_pool = ctx.enter_context(tc.tile_pool(name="vp", bufs=2, space="PSUM"))
    op_pool = ctx.enter_context(tc.tile_pool(name="op", bufs=2, space="PSUM"))
    h_pool = ctx.enter_context(tc.tile_pool(name="h", bufs=2))
    bb_pool = ctx.enter_context(tc.tile_pool(name="bb", bufs=3))
    tr_pool = ctx.enter_context(tc.tile_pool(name="tr", bufs=6))
    words_pool = ctx.enter_context(tc.tile_pool(name="words", bufs=10))
    conf_pool = ctx.enter_context(tc.tile_pool(name="conf", bufs=10))
    c2_pool = ctx.enter_context(tc.tile_pool(name="c2", bufs=4))
    g_pool = ctx.enter_context(tc.tile_pool(name="g", bufs=8))
    vsb_pool = ctx.enter_context(tc.tile_pool(name="vsb", bufs=2))
    osb_pool = ctx.enter_context(tc.tile_pool(name="osb", bufs=4))
    c_pool = ctx.enter_context(tc.tile_pool(name="cc", bufs=2))

    x_sb = const.tile([128, NLOC, C, H], f32)
    nc.sync.dma_start(x_sb[:], xt_d[:])
    ipm_sb = const.tile([128, 18, HP], f32)
    nc.sync.dma_start(ipm_sb[:], ipm_d[:])
    bx_sb = const.tile([HP, PO], f32)
    nc.sync.dma_start(bx_sb[:], bx_d[:])
    i120_sb = const.tile([HP, HP], f16)
    nc.sync.dma_start(i120_sb[:], i120_d[:])
    thrb_sb = const.tile([128, M * K + 1], f32)
    nc.sync.dma_start(thrb_sb[:], thrb_d[:])
    ones_sb = const.tile([1, HP], f32)
    nc.vector.memset(ones_sb[:], 1.0)

    FR = NG * HP  # free size of a stage-A plane (240), layout (n, y)

    for g in range(GROUPS):
        nsl = slice(g * NG, (g + 1) * NG)
        w_tiles = []
        conf_tiles = []
        for m in range(M):
            h_t = h_pool.tile([HP, K, FR], f16)
            for k in range(K):
                c1, c2 = int(chan_idx[m, k, 0]), int(chan_idx[m, k, 1])
                dy1, dx1 = int(offsets[m, k, 0, 0]), int(offsets[m, k, 0, 1])
                dy2, dx2 = int(offsets[m, k, 1, 0]), int(offsets[m, k, 1, 1])
                zp = zp_pool.tile([HP, FR], f32)
                nc.tensor.matmul(
                    zp[:], lhsT=ipm_sb[:, dx1, :],
                    rhs=x_sb[:, nsl, c1, dy1 : dy1 + HP],
                    start=True, stop=False,
                )
                nc.tensor.matmul(
                    zp[:], lhsT=ipm_sb[:, 9 + dx2, :],
                    rhs=x_sb[:, nsl, c2, dy2 : dy2 + HP],
                    start=False, stop=False,
                )
                # exact -thr accumulation: matches the reference's fp32
                # rounding of (x1 - x2) - thr, so bit decisions agree
                nc.tensor.matmul(
                    zp[:], lhsT=ones_sb[:],
                    rhs=thrb_sb[0:1, m * K + k : m * K + k + 1].to_broadcast([1, FR]),
                    start=False, stop=True,
                )
                nc.scalar.activation(
                    h_t[:, k, :], zp[:], Act.Tanh,
                    bias=thrb_sb[:HP, M * K : M * K + 1], scale=5.0,
                )
            # word = sum_k 2^k [h_k > 0] via PE accumulation
            wp = wp_pool.tile([HP, FR], f32)
            for k in range(K):
                b_t = bb_pool.tile([HP, FR], f16)
                nc.vector.tensor_scalar(
                    b_t[:], h_t[:, k, :], 0.0, float(2 ** k), Alu.is_gt, Alu.mult
                )
                nc.tensor.matmul(
                    wp[:], lhsT=i120_sb[:], rhs=b_t[:],
                    start=(k == 0), stop=(k == K - 1),
                )
            w_t = words_pool.tile([HP, FR], i32)
            nc.vector.tensor_copy(w_t[:], wp[:])
            w_tiles.append(w_t)
            # conf = prod_k (1 + |h_k|):  |h| = max(-h, h) via STT, then
            # conf <- (|h| + 1) * conf fused in one STT per bit.
            conf_t = conf_pool.tile([HP, FR], f16)
            t0 = tr_pool.tile([HP, FR], f16, tag="tr")
            nc.vector.scalar_tensor_tensor(
                t0[:], h_t[:, 0, :], -1.0, h_t[:, 0, :], Alu.mult, Alu.max
            )
            nc.vector.tensor_scalar(conf_t[:], t0[:], 1.0, None, Alu.add)
            for k in range(1, K):
                u_t = tr_pool.tile([HP, FR], f16, tag="tr")
                nc.vector.scalar_tensor_tensor(
                    u_t[:], h_t[:, k, :], -1.0, h_t[:, k, :], Alu.mult, Alu.max
                )
                nc.vector.scalar_tensor_tensor(
                    conf_t[:], u_t[:], 1.0, conf_t[:], Alu.add, Alu.mult
                )
            conf_tiles.append(conf_t)

        for i in range(NG):
            n_loc = g * NG + i
            isl = slice(i * HP, (i + 1) * HP)
            g_tiles = []
            for m in range(M):
                gt = g_pool.tile([HP, HP, D], f16)
                wsl = w_tiles[m]
                for y in range(HP):
                    nc.gpsimd.indirect_dma_start(
                        out=gt[:, y, :],
                        out_offset=None,
                        in_=tbl_d,
                        in_offset=bass.IndirectOffsetOnAxis(
                            ap=wsl[:, i * HP + y : i * HP + y + 1], axis=0
                        ),
                        element_offset=m * 4096 * D,
                    )
                # duplicate conf into adjacent pairs for 2x-packed weighting
                c2_t = c2_pool.tile([HP, HP, 2], f16)
                nc.vector.tensor_copy(c2_t[:, :, 0], conf_tiles[m][:, isl])
                nc.vector.tensor_copy(c2_t[:, :, 1], conf_tiles[m][:, isl])
                gap = gt[:]
                gv = bass.AP(gap.tensor, gap.offset,
                             [gap.ap[0], [D, HP], [2, D // 2], [1, 2]])
                cap = c2_t[:]
                cv = bass.AP(cap.tensor, cap.offset,
                             [cap.ap[0], [2, HP], [0, D // 2], [1, 2]])
                nc.vector.tensor_tensor(gv, gv, cv, Alu.mult)
                g_tiles.append(gt)
            # fern-accumulate into PSUM; drain d-major into vsb [x, d, y]
            vsb = vsb_pool.tile([HP, D, HP], f16)
            ylen = HP // YCH
            for ch in range(YCH):
                vp = vp_pool.tile([HP, ylen * D], f32)
                for m in range(M):
                    nc.tensor.matmul(
                        vp[:],
                        lhsT=i120_sb[:],
                        rhs=g_tiles[m][:, ch * ylen : (ch + 1) * ylen, :],
                        start=(m == 0), stop=(m == M - 1),
                    )
                # vp iterates (y, d); write transposed into vsb (d-major)
                vap = vsb[:]
                v_out = bass.AP(vap.tensor, vap.offset + ch * ylen,
                                [vap.ap[0], [1, ylen], [HP, D]])
                nc.scalar.copy(v_out, vp[:])
            # y-pooling: plain running cumsum over the whole (d, y) stream;
            # the carry across d-blocks cancels in the windowed difference.
            c_t = c_pool.tile([HP, D * HP + 1], f32)
            nc.vector.memset(c_t[:, 0:1], 0.0)
            vflat = vsb[:]
            v2d = bass.AP(vflat.tensor, vflat.offset, [vflat.ap[0], [1, D * HP]])
            nc.vector.tensor_tensor_scan(
                c_t[:, 1 : D * HP + 1], v2d, v2d, 0.0, Alu.add, Alu.bypass
            )
            # Dy[d, y'] = C[d*120+y'+7] - C[d*120+y'], written in place at
            # c_t[:, d*120+y']  (write trails reads by 7 -> safe)
            cap = c_t[:]
            c_hi = bass.AP(cap.tensor, cap.offset + 7, [cap.ap[0], [HP, D], [1, PO]])
            c_lo = bass.AP(cap.tensor, cap.offset, [cap.ap[0], [HP, D], [1, PO]])
            nc.vector.tensor_tensor(c_lo, c_hi, c_lo, Alu.subtract)
            # x-pooling + transpose: out[y', x'] = sum_x Dy[x, y'] * bx[x, x']
            for d in range(D):
                op_t = op_pool.tile([PO, PO], f32)
                nc.tensor.matmul(
                    op_t[:], lhsT=c_t[:, d * HP : d * HP + PO], rhs=bx_sb[:],
                    start=True, stop=True,
                )
                o_sb = osb_pool.tile([PO, PO], f32)
                nc.scalar.copy(o_sb[:], op_t[:])
                nc.sync.dma_start(out_d[n_loc, d], o_sb[:])


def _compile(thresholds, chan_idx, offsets):
    key = (thresholds.tobytes(), chan_idx.tobytes(), offsets.tobytes())
    if _CACHE.get("key") == key:
        return _CACHE["nc"]
    from concourse import bacc
    import concourse.tile as tile

    nc = bacc.Bacc("TRN2", target_bir_lowering=False, debug=False)
    with tile.TileContext(nc) as tc:
        with ExitStack() as ctx:
            _build(ctx, nc, tc, thresholds, chan_idx, offsets)
    nc.compile()
    _CACHE["key"] = key
    _CACHE["nc"] = nc
    return nc


def _install_ntff_hook():
    """Recreate the antenv.axon_hooks NTFF-profile hook this image lacks."""
    import types
    import antenv

    if getattr(antenv, "axon_hooks", None) is not None:
        return
    mod = types.ModuleType("antenv.axon_hooks")
    holder = [None]
    mod.set_axon_ntff_profile_hook = lambda h: holder.__setitem__(0, h)
    mod.get_axon_ntff_profile_hook = lambda: holder[0]
    sys.modules["antenv.axon_hooks"] = mod
    antenv.axon_hooks = mod
    try:
        if "/root/.axon_site" not in sys.path:
            sys.path.insert(0, "/root/.axon_site")
        from trn_agent_boot.trn_boot import _ntff_profile_via_ctypes

        holder[0] = _ntff_profile_via_ctypes("/opt/axon/libaxon_pjrt.so")
    except Exception:
        holder[0] = None


def _run(x, thresholds, table, chan_idx, offsets, trace=False):
    from concourse.bass_utils import run_bass_kernel_spmd

    if trace:
        try:
            _install_ntff_hook()
        except Exception:
            pass

    x = np.asarray(x, np.float32)
    thresholds = np.asarray(thresholds, np.float32)
    table = np.asarray(table, np.float32)
    chan_idx = np.asarray(chan_idx)
    offsets = np.asarray(offsets)

    tbl16, bx16, ipm, i120 = _host_constants(table)
    thrb = _host_biases(thresholds)
    xt = np.ascontiguousarray(x.transpose(3, 0, 1, 2))  # [W, N, C, H]

    nc = _compile(thresholds, chan_idx, offsets)

    in_maps = []
    for c in range(NCORES):
        in_maps.append({
            "xt": np.ascontiguousarray(xt[:, c * NLOC : (c + 1) * NLOC]),
            "tbl": tbl16,
            "bx": bx16,
            "ipm": ipm,
            "i120": i120,
            "thrb": thrb,
        })
    res = run_bass_kernel_spmd(nc, in_maps, core_ids=list(range(NCORES)), trace=trace)
    out = np.concatenate([res.results[c]["out"].reshape(NLOC, -1) for c in range(NCORES)], 0)
    return out.astype(np.float32), res


def kernel(x, thresholds, table, chan_idx, offsets):
    out, _ = _run(x, thresholds, table, chan_idx, offsets)
    return out



# revision 6
# speedup vs baseline: 1.5166x; 1.5166x over previous
"""Trainium2 Bass kernel for the random-fern VQ-codebook problem (nn_CTE_37512244364031).

v3: data-parallel over batch N across 8 NeuronCores (8 images/core), with the
per-pixel table lookup done by gpsimd.ap_gather from an SBUF-resident table
(no per-row SWDGE DMAs, which dominated the old kernel at ~8.7ms/core).

Pixel slot scheme (per image): slot j = c*16 + p16 with c = xb*120 + y,
pixel (x = 8*p16 + xb, y); x in [120,128) (p16 == 15) is junk, never read
by pooling. The wrapped ap_gather index layout [16m+p16, c] is produced from
the stage-A-native word tile [x, (n,y)] by a pure partition-fold DMA
(flat orders match exactly).

Per core:
  stage A (per fern m, per bit k), layout [x=128 partitions, (n4, y120)]:
    - diff via 2 PE matmuls with +/- shifted identities (fp32, exact)
    - ACT tanh(5 z - 5 thr) via per-plane bias column -> h fp16
    - word = sum_k 2^k (h_k > 0): DVE bits + PE identity-matmul accumulation,
      drained to int16 (ap_gather indices)
    - conf = prod_k (1 + |h_k|) via DVE STT chain (2^-12 folded into scales)
  stage B (per image):
    - fold DMAs: words/conf [x, y] -> wrapped [16m+p16, (xb, y)]
    - ap_gather: g[16m+dblk, slot, 2] = table[m, w_m(slot), 2dblk:2dblk+2]
    - conf broadcast to 128 partitions via 16 PE selection matmuls
    - wg = g * conf (DVE, stride-0 pair-dup)
    - m-reduce: 2 e-pass PE matmuls (lhsT R2) -> votes [32 d, slot] PSUM,
      drained fp16 with the 2^-6/49 scale
    - pooling: 7-tap y-adds into x-contiguous layout, then 7-tap x-adds
    - out fp16 [d, y', x'] DMA'd per image; host casts to f32
"""

import os
import sys
from contextlib import ExitStack

import numpy as np

sys.path.insert(0, "/opt/trn_rl_repo")

M, K, L, D = 8, 12, 9, 32
N, C, H, W = 64, 8, 128, 128
HP = WP = 120
PO = 114
NCORES = 8
NLOC = N // NCORES          # 8 images per core
NG = 4                      # images per stage-A group
GROUPS = NLOC // NG
FR = NG * HP                # 480
XP = 128                    # padded x extent
P16 = 16
NXB = 8                     # xb blocks (x = 8*p16 + xb)
CI = NXB * HP               # 960 c-columns per image
QI = CI * P16               # 15360 slots per image
CCH = 120                   # c-columns per gather chunk
QCH = CCH * P16             # 1920 slots per chunk
NCH = CI // CCH             # 8 chunks
SUB = 480                   # slots per m-reduce PSUM chunk
TSCALE = 2.0 ** -6          # folded into table
BSCALE = 2.0 ** -6 / 49.0   # applied at the votes drain

_CACHE = {}


def _host_constants(table):
    # tblr[16m+dblk, w, e] = table[m, w, 2*dblk+e] * TSCALE
    tblr = np.empty((128, 4096, 2), np.float16)
    for m in range(M):
        for dblk in range(16):
            tblr[16 * m + dblk] = (table[m][:, 2 * dblk : 2 * dblk + 2] * TSCALE)
    tblr = np.ascontiguousarray(tblr)

    # shifted +/- identities: ipm[j+s, s, j] = +1, ipm[j+s, 9+s, j] = -1
    ipm = np.zeros((128, 18, XP), np.float32)
    for s in range(9):
        for j in range(XP):
            if j + s < 128:
                ipm[j + s, s, j] = 1.0
                ipm[j + s, 9 + s, j] = -1.0

    i128 = np.eye(128, dtype=np.float16)

    # L16[16m+s, s, 16m+d16] = 1  (conf partition-broadcast selectors)
    l16 = np.zeros((128, P16, 128), np.float16)
    for m in range(M):
        for s in range(P16):
            for d16 in range(P16):
                l16[16 * m + s, s, 16 * m + d16] = 1.0

    # R2[16m+dblk, e, 2dblk+e] = 1  (m-reduce / d-unzip selectors)
    r2 = np.zeros((128, 2, D), np.float16)
    for m in range(M):
        for dblk in range(P16):
            for e in range(2):
                r2[16 * m + dblk, e, 2 * dblk + e] = 1.0

    return tblr, ipm, i128, l16, r2


def _host_biases(thresholds):
    # -5*thr per (m,k) + a zeros column, replicated across partitions
    b = np.zeros((1, M * K + 1), np.float32)
    b[0, : M * K] = -5.0 * thresholds.reshape(M * K)
    return np.ascontiguousarray(np.repeat(b, 128, axis=0))


def _build(ctx, nc, tc, thresholds, chan_idx, offsets):
    import concourse.bass as bass
    from concourse import mybir

    f32 = mybir.dt.float32
    f16 = mybir.dt.float16
    i16 = mybir.dt.int16
    Alu = mybir.AluOpType
    Act = mybir.ActivationFunctionType

    xt_d = nc.dram_tensor("xt", [128, NLOC, C, H], f32, kind="ExternalInput").ap()
    tblr_d = nc.dram_tensor("tblr", [128, 4096, 2], f16, kind="ExternalInput").ap()
    ipm_d = nc.dram_tensor("ipm", [128, 18, XP], f32, kind="ExternalInput").ap()
    i128_d = nc.dram_tensor("i128", [128, 128], f16, kind="ExternalInput").ap()
    l16_d = nc.dram_tensor("l16", [128, P16, 128], f16, kind="ExternalInput").ap()
    r2_d = nc.dram_tensor("r2", [128, 2, D], f16, kind="ExternalInput").ap()
    thrb_d = nc.dram_tensor("thrb", [128, M * K + 1], f32, kind="ExternalInput").ap()
    out_d = nc.dram_tensor("out", [NLOC, D, PO, PO], f16, kind="ExternalOutput").ap()

    const = ctx.enter_context(tc.tile_pool(name="const", bufs=1))
    zp_pool = ctx.enter_context(tc.tile_pool(name="zp", bufs=2, space="PSUM"))
    wp_pool = ctx.enter_context(tc.tile_pool(name="wp", bufs=2, space="PSUM"))
    cp_pool = ctx.enter_context(tc.tile_pool(name="cp", bufs=2, space="PSUM"))
    vp_pool = ctx.enter_context(tc.tile_pool(name="vp", bufs=2, space="PSUM"))
    h_pool = ctx.enter_context(tc.tile_pool(name="h", bufs=1))
    bb_pool = ctx.enter_context(tc.tile_pool(name="bb", bufs=3))
    words_pool = ctx.enter_context(tc.tile_pool(name="words", bufs=12))
    conf_pool = ctx.enter_context(tc.tile_pool(name="confp", bufs=12))
    idxw_pool = ctx.enter_context(tc.tile_pool(name="idxw", bufs=2))
    confw_pool = ctx.enter_context(tc.tile_pool(name="confw", bufs=2))
    g_pool = ctx.enter_context(tc.tile_pool(name="g", bufs=2))
    cbc_pool = ctx.enter_context(tc.tile_pool(name="cbc", bufs=2))
    vsb_pool = ctx.enter_context(tc.tile_pool(name="vsb", bufs=1))
    dyx_pool = ctx.enter_context(tc.tile_pool(name="dyx", bufs=1))
    po_pool = ctx.enter_context(tc.tile_pool(name="po", bufs=1))

    x_sb = const.tile([128, NLOC, C, H], f32)
    nc.sync.dma_start(x_sb[:], xt_d[:])
    tblr_sb = const.tile([128, 4096, 2], f16)
    nc.sync.dma_start(tblr_sb[:], tblr_d[:])
    ipm_sb = const.tile([128, 18, XP], f32)
    nc.sync.dma_start(ipm_sb[:], ipm_d[:])
    i128_sb = const.tile([128, 128], f16)
    nc.sync.dma_start(i128_sb[:], i128_d[:])
    l16_sb = const.tile([128, P16, 128], f16)
    nc.sync.dma_start(l16_sb[:], l16_d[:])
    r2_sb = const.tile([128, 2, D], f16)
    nc.sync.dma_start(r2_sb[:], r2_d[:])
    thrb_sb = const.tile([128, M * K + 1], f32)
    nc.sync.dma_start(thrb_sb[:], thrb_d[:])

    for g in range(GROUPS):
        nsl = slice(g * NG, (g + 1) * NG)
        wt_tiles = []
        conf_tiles = []
        for m in range(M):
            h_t = h_pool.tile([128, K, FR], f16)
            for k in range(K):
                c1, c2 = int(chan_idx[m, k, 0]), int(chan_idx[m, k, 1])
                dy1, dx1 = int(offsets[m, k, 0, 0]), int(offsets[m, k, 0, 1])
                dy2, dx2 = int(offsets[m, k, 1, 0]), int(offsets[m, k, 1, 1])
                mk = m * K + k
                zp = zp_pool.tile([128, FR], f32)
                nc.tensor.matmul(
                    zp[:], lhsT=ipm_sb[:, dx1, :],
                    rhs=x_sb[:, nsl, c1, dy1 : dy1 + HP],
                    start=True, stop=False,
                )
                nc.tensor.matmul(
                    zp[:], lhsT=ipm_sb[:, 9 + dx2, :],
                    rhs=x_sb[:, nsl, c2, dy2 : dy2 + HP],
                    start=False, stop=True,
                )
                nc.scalar.activation(
                    h_t[:, k, :], zp[:], Act.Tanh,
                    bias=thrb_sb[:, mk : mk + 1], scale=5.0,
                )
            # word = sum_k 2^k [h_k > 0] via PE accumulation; drain to int16
            wp = wp_pool.tile([128, FR], f32)
            for k in range(K):
                b_t = bb_pool.tile([128, FR], f16)
                nc.vector.tensor_scalar(
                    b_t[:], h_t[:, k, :], 0.0, float(2 ** k), Alu.is_gt, Alu.mult
                )
                nc.tensor.matmul(
                    wp[:], lhsT=i128_sb[:], rhs=b_t[:],
                    start=(k == 0), stop=(k == K - 1),
                )
            wt = words_pool.tile([128, FR], i16)
            nc.vector.tensor_copy(wt[:], wp[:])
            wt_tiles.append(wt)
            # conf = prod_k (1 + |h_k|)
            conf_t = conf_pool.tile([128, FR], f16)
            t0 = bb_pool.tile([128, FR], f16)
            nc.vector.scalar_tensor_tensor(
                t0[:], h_t[:, 0, :], -1.0, h_t[:, 0, :], Alu.mult, Alu.max
            )
            nc.vector.tensor_scalar(conf_t[:], t0[:], 1.0, None, Alu.add)
            for k in range(1, K):
                u_t = bb_pool.tile([128, FR], f16)
                nc.vector.scalar_tensor_tensor(
                    u_t[:], h_t[:, k, :], -1.0, h_t[:, k, :], Alu.mult, Alu.max
                )
                nc.vector.scalar_tensor_tensor(
                    conf_t[:], u_t[:], 1.0, conf_t[:], Alu.add, Alu.mult
                )
            conf_tiles.append(conf_t)

        for ii in range(NG):
            i_loc = g * NG + ii
            isl = slice(ii * HP, (ii + 1) * HP)
            # partition-fold: [x=8*p16+xb, y] -> [16m+p16, c=(xb,y)]
            idxw = idxw_pool.tile([128, CI], i16)
            confw = confw_pool.tile([128, CI], f16)
            for m in range(M):
                nc.sync.dma_start(idxw[16 * m : 16 * m + 16, :], wt_tiles[m][:, isl])
                nc.sync.dma_start(
                    confw[16 * m : 16 * m + 16, :], conf_tiles[m][:, isl]
                )
            vsb = vsb_pool.tile([32, QI], f16)
            for ch in range(NCH):
                csl = slice(ch * CCH, (ch + 1) * CCH)
                g_t = g_pool.tile([128, QCH, 2], f16)
                nc.gpsimd.ap_gather(
                    out_ap=g_t[:], in_ap=tblr_sb[:], idxs_ap=idxw[:, csl],
                    channels=128, num_elems=4096, d=2, num_idxs=QCH,
                )
                # conf broadcast [16m+p16] -> [16m+dblk] for each p16
                cbc = cbc_pool.tile([128, CCH, P16], f16)
                for s in range(P16):
                    cp = cp_pool.tile([128, CCH], f32)
                    nc.tensor.matmul(
                        cp[:], lhsT=l16_sb[:, s, :], rhs=confw[:, csl],
                        start=True, stop=True,
                    )
                    nc.vector.tensor_copy(cbc[:, :, s], cp[:])
                # wg = g * conf (pair-dup via stride-0)
                gap = g_t[:]
                cap = cbc[:]
                cv = bass.AP(cap.tensor, cap.offset,
                             [cap.ap[0], [1, QCH], [0, 2]])
                nc.vector.tensor_tensor(gap, gap, cv, Alu.mult)
                # m-reduce + d-unzip: votes[2*dblk+e, slot] = sum_m wg[(m,dblk), slot, e]
                for sub in range(QCH // SUB):
                    vp = vp_pool.tile([32, SUB], f32)
                    for e in range(2):
                        rv = bass.AP(gap.tensor, gap.offset + sub * SUB * 2 + e,
                                     [gap.ap[0], [2, SUB]])
                        nc.tensor.matmul(
                            vp[:], lhsT=r2_sb[:, e, :], rhs=rv,
                            start=(e == 0), stop=(e == 1),
                        )
                    nc.scalar.activation(
                        vsb[:, ch * QCH + sub * SUB : ch * QCH + (sub + 1) * SUB],
                        vp[:], Act.Copy, bias=0.0, scale=BSCALE,
                    )
            # pooling: slot = xb*1920 + y*16 + p16; pixel (x = 8*p16+xb, y)
            dyx = dyx_pool.tile([32, PO, XP], f16)
            vap = vsb[:]
            dap = dyx[:]
            dst = bass.AP(dap.tensor, dap.offset,
                          [dap.ap[0], [1, NXB], [XP, PO], [NXB, P16]])
            for t in range(7):
                src = bass.AP(vap.tensor, vap.offset + t * P16,
                              [vap.ap[0], [P16 * HP, NXB], [P16, PO], [1, P16]])
                if t == 0:
                    nc.vector.tensor_copy(dst, src)
                else:
                    nc.vector.tensor_tensor(dst, src, dst, Alu.add)
            for sl, (y0, y1) in enumerate(((0, 57), (57, PO))):
                po = po_pool.tile([32, 57, PO], f16)
                for t in range(7):
                    src = dyx[:, y0:y1, t : t + PO]
                    if t == 0:
                        nc.vector.tensor_copy(po[:], src)
                    else:
                        nc.vector.tensor_tensor(po[:], src, po[:], Alu.add)
                nc.sync.dma_start(out_d[i_loc, :, y0:y1, :], po[:])


def _compile(thresholds, chan_idx, offsets):
    key = (thresholds.tobytes(), chan_idx.tobytes(), offsets.tobytes())
    if _CACHE.get("key") == key:
        return _CACHE["nc"]
    from concourse import bacc
    import concourse.tile as tile

    nc = bacc.Bacc("TRN2", target_bir_lowering=False, debug=False)
    with tile.TileContext(nc) as tc:
        with ExitStack() as ctx:
            _build(ctx, nc, tc, thresholds, chan_idx, offsets)
    nc.compile()
    _CACHE["key"] = key
    _CACHE["nc"] = nc
    return nc


def _install_ntff_hook():
    """Recreate the antenv.axon_hooks NTFF-profile hook this image lacks."""
    import types
    import antenv

    if getattr(antenv, "axon_hooks", None) is not None:
        return
    mod = types.ModuleType("antenv.axon_hooks")
    holder = [None]
    mod.set_axon_ntff_profile_hook = lambda h: holder.__setitem__(0, h)
    mod.get_axon_ntff_profile_hook = lambda: holder[0]
    sys.modules["antenv.axon_hooks"] = mod
    antenv.axon_hooks = mod
    try:
        if "/root/.axon_site" not in sys.path:
            sys.path.insert(0, "/root/.axon_site")
        from trn_agent_boot.trn_boot import _ntff_profile_via_ctypes

        holder[0] = _ntff_profile_via_ctypes("/opt/axon/libaxon_pjrt.so")
    except Exception:
        holder[0] = None


def _run(x, thresholds, table, chan_idx, offsets, trace=False):
    from concourse.bass_utils import run_bass_kernel_spmd

    if trace:
        try:
            _install_ntff_hook()
        except Exception:
            pass

    x = np.asarray(x, np.float32)
    thresholds = np.asarray(thresholds, np.float32)
    table = np.asarray(table, np.float32)
    chan_idx = np.asarray(chan_idx)
    offsets = np.asarray(offsets)

    tblr, ipm, i128, l16, r2 = _host_constants(table)
    thrb = _host_biases(thresholds)
    xt = np.ascontiguousarray(x.transpose(3, 0, 1, 2))  # [W, N, C, H]

    nc = _compile(thresholds, chan_idx, offsets)

    in_maps = []
    for c in range(NCORES):
        in_maps.append({
            "xt": np.ascontiguousarray(xt[:, c * NLOC : (c + 1) * NLOC]),
            "tblr": tblr,
            "ipm": ipm,
            "i128": i128,
            "l16": l16,
            "r2": r2,
            "thrb": thrb,
        })
    res = run_bass_kernel_spmd(nc, in_maps, core_ids=list(range(NCORES)), trace=trace)
    out = np.concatenate(
        [res.results[c]["out"].astype(np.float32).reshape(NLOC, -1)
         for c in range(NCORES)], 0
    )
    return out, res


def kernel(x, thresholds, table, chan_idx, offsets):
    out, _ = _run(x, thresholds, table, chan_idx, offsets)
    return out


# revision 12
# speedup vs baseline: 2.5002x; 1.6486x over previous
"""Trainium2 Bass kernel for the random-fern VQ-codebook problem (nn_CTE_37512244364031).

v3: data-parallel over batch N across 8 NeuronCores (8 images/core), with the
per-pixel table lookup done by gpsimd.ap_gather from an SBUF-resident table
(no per-row SWDGE DMAs, which dominated the old kernel at ~8.7ms/core).

Pixel slot scheme (per image): slot j = c*16 + p16 with c = xb*120 + y,
pixel (x = 8*p16 + xb, y); x in [120,128) (p16 == 15) is junk, never read
by pooling. The wrapped ap_gather index layout [16m+p16, c] is produced from
the stage-A-native word tile [x, (n,y)] by a pure partition-fold DMA
(flat orders match exactly).

Per core:
  stage A (per fern m, per bit k), layout [x=128 partitions, (n4, y120)]:
    - diff via 2 PE matmuls with +/- shifted identities (fp32, exact)
    - ACT tanh(5 z - 5 thr) via per-plane bias column -> h fp16
    - word = sum_k 2^k (h_k > 0): DVE bits + PE identity-matmul accumulation,
      drained to int16 (ap_gather indices)
    - conf = prod_k (1 + |h_k|) via DVE STT chain (2^-12 folded into scales)
  stage B (per image):
    - fold DMAs: words/conf [x, y] -> wrapped [16m+p16, (xb, y)]
    - ap_gather: g[16m+dblk, slot, 2] = table[m, w_m(slot), 2dblk:2dblk+2]
    - conf broadcast to 128 partitions via 16 PE selection matmuls
    - wg = g * conf (DVE, stride-0 pair-dup)
    - m-reduce: 2 e-pass PE matmuls (lhsT R2) -> votes [32 d, slot] PSUM,
      drained fp16 with the 2^-6/49 scale
    - pooling: 7-tap y-adds into x-contiguous layout, then 7-tap x-adds
    - out fp16 [d, y', x'] DMA'd per image; host casts to f32
"""

import os
import sys
from contextlib import ExitStack

import numpy as np

sys.path.insert(0, "/opt/trn_rl_repo")

M, K, L, D = 8, 12, 9, 32
N, C, H, W = 64, 8, 128, 128
HP = WP = 120
PO = 114
NCORES = 8
NLOC = N // NCORES          # 8 images per core
NG = 2                      # images per stage-A group
GROUPS = NLOC // NG
FR = NG * HP                # 480
XP = 128                    # padded x extent
P16 = 16
NXB = 8                     # xb blocks (x = 8*p16 + xb)
CI = NXB * HP               # 960 c-columns per image
QI = CI * P16               # 15360 slots per image
CCH = 120                   # c-columns per gather chunk
QCH = CCH * P16             # 1920 slots per chunk
NCH = CI // CCH             # 8 chunks
SUB = 480                   # slots per m-reduce PSUM chunk
TSCALE = 2.0 ** -6          # folded into table
BSCALE = 2.0 ** -6 / 49.0   # applied at the votes drain

_CACHE = {}


def _host_constants(table):
    # tblr[16m+dblk, w, e] = table[m, w, 2*dblk+e] * TSCALE
    tblr = np.empty((128, 4096, 2), np.float16)
    for m in range(M):
        for dblk in range(16):
            tblr[16 * m + dblk] = (table[m][:, 2 * dblk : 2 * dblk + 2] * TSCALE)
    tblr = np.ascontiguousarray(tblr)

    # shifted +/- identities: ipm[j+s, s, j] = +1, ipm[j+s, 9+s, j] = -1
    ipm = np.zeros((128, 18, XP), np.float32)  # cast to bf16 at the end
    for s in range(9):
        for j in range(XP):
            if j + s < 128:
                ipm[j + s, s, j] = 1.0
                ipm[j + s, 9 + s, j] = -1.0

    i128 = np.eye(128, dtype=np.float16)

    # L16[16m+s, s, 16m+d16] = 1  (conf partition-broadcast selectors)
    l16 = np.zeros((128, P16, 128), np.float16)
    for m in range(M):
        for s in range(P16):
            for d16 in range(P16):
                l16[16 * m + s, s, 16 * m + d16] = 1.0

    # R2[16m+dblk, e, 2dblk+e] = 1  (m-reduce / d-unzip selectors)
    r2 = np.zeros((128, 2, D), np.float16)
    for m in range(M):
        for dblk in range(P16):
            for e in range(2):
                r2[16 * m + dblk, e, 2 * dblk + e] = 1.0

    return tblr, ipm, i128, l16, r2


def _host_biases(thresholds):
    # -5*thr per (m,k) + a zeros column, replicated across partitions
    b = np.zeros((1, M * K + 1), np.float32)
    b[0, : M * K] = -5.0 * thresholds.reshape(M * K)
    return np.ascontiguousarray(np.repeat(b, 128, axis=0))


def _build(ctx, nc, tc, thresholds, chan_idx, offsets):
    import concourse.bass as bass
    from concourse import mybir

    f32 = mybir.dt.float32
    f16 = mybir.dt.float16
    i16 = mybir.dt.int16
    Alu = mybir.AluOpType
    Act = mybir.ActivationFunctionType

    xt_d = nc.dram_tensor("xt", [128, NLOC, C, H], f32, kind="ExternalInput").ap()
    tblr_d = nc.dram_tensor("tblr", [128, 4096, 2], f16, kind="ExternalInput").ap()
    ipm_d = nc.dram_tensor("ipm", [128, 18, XP], f32, kind="ExternalInput").ap()
    i128_d = nc.dram_tensor("i128", [128, 128], f16, kind="ExternalInput").ap()
    l16_d = nc.dram_tensor("l16", [128, P16, 128], f16, kind="ExternalInput").ap()
    r2_d = nc.dram_tensor("r2", [128, 2, D], f16, kind="ExternalInput").ap()
    thrb_d = nc.dram_tensor("thrb", [128, M * K + 1], f32, kind="ExternalInput").ap()
    out_d = nc.dram_tensor("out", [NLOC, D, PO, PO], f16, kind="ExternalOutput").ap()

    const = ctx.enter_context(tc.tile_pool(name="const", bufs=1))
    zp_pool = ctx.enter_context(tc.tile_pool(name="zp", bufs=2, space="PSUM"))
    wp_pool = ctx.enter_context(tc.tile_pool(name="wp", bufs=2, space="PSUM"))
    cp_pool = ctx.enter_context(tc.tile_pool(name="cp", bufs=2, space="PSUM"))
    vp_pool = ctx.enter_context(tc.tile_pool(name="vp", bufs=2, space="PSUM"))
    h_pool = ctx.enter_context(tc.tile_pool(name="h", bufs=1))
    bb_pool = ctx.enter_context(tc.tile_pool(name="bb", bufs=2))
    words_pool = ctx.enter_context(tc.tile_pool(name="words", bufs=12))
    conf_pool = ctx.enter_context(tc.tile_pool(name="confp", bufs=12))
    idxw_pool = ctx.enter_context(tc.tile_pool(name="idxw", bufs=1))
    confw_pool = ctx.enter_context(tc.tile_pool(name="confw", bufs=1))
    g_pool = ctx.enter_context(tc.tile_pool(name="g", bufs=2))
    cbc_pool = ctx.enter_context(tc.tile_pool(name="cbc", bufs=2))
    vsb_pool = ctx.enter_context(tc.tile_pool(name="vsb", bufs=1))
    dyx_pool = ctx.enter_context(tc.tile_pool(name="dyx", bufs=1))
    po_pool = ctx.enter_context(tc.tile_pool(name="po", bufs=1))

    x_sb = const.tile([128, NLOC, C, H], f32)
    nc.sync.dma_start(x_sb[:], xt_d[:])
    tblr_sb = const.tile([128, 4096, 2], f16)
    nc.sync.dma_start(tblr_sb[:], tblr_d[:])
    ipm_sb = const.tile([128, 18, XP], f32)
    nc.sync.dma_start(ipm_sb[:], ipm_d[:])
    i128_sb = const.tile([128, 128], f16)
    nc.sync.dma_start(i128_sb[:], i128_d[:])
    l16_sb = const.tile([128, P16, 128], f16)
    nc.sync.dma_start(l16_sb[:], l16_d[:])
    r2_sb = const.tile([128, 2, D], f16)
    nc.sync.dma_start(r2_sb[:], r2_d[:])
    thrb_sb = const.tile([128, M * K + 1], f32)
    nc.sync.dma_start(thrb_sb[:], thrb_d[:])

    for g in range(GROUPS):
        nsl = slice(g * NG, (g + 1) * NG)
        wt_tiles = []
        conf_tiles = []
        for m in range(M):
            h_t = h_pool.tile([128, K, FR], f16)
            for k in range(K):
                c1, c2 = int(chan_idx[m, k, 0]), int(chan_idx[m, k, 1])
                dy1, dx1 = int(offsets[m, k, 0, 0]), int(offsets[m, k, 0, 1])
                dy2, dx2 = int(offsets[m, k, 1, 0]), int(offsets[m, k, 1, 1])
                mk = m * K + k
                zp = zp_pool.tile([128, FR], f32)
                nc.tensor.matmul(
                    zp[:], lhsT=ipm_sb[:, dx1, :],
                    rhs=x_sb[:, nsl, c1, dy1 : dy1 + HP],
                    start=True, stop=False,
                )
                nc.tensor.matmul(
                    zp[:], lhsT=ipm_sb[:, 9 + dx2, :],
                    rhs=x_sb[:, nsl, c2, dy2 : dy2 + HP],
                    start=False, stop=True,
                )
                nc.scalar.activation(
                    h_t[:, k, :], zp[:], Act.Tanh,
                    bias=thrb_sb[:, mk : mk + 1], scale=5.0,
                )
            # word = sum_k 2^k [h_k > 0] via PE accumulation; drain to int16
            wp = wp_pool.tile([128, FR], f32)
            for k in range(K):
                b_t = bb_pool.tile([128, FR], f16)
                nc.vector.tensor_scalar(
                    b_t[:], h_t[:, k, :], 0.0, float(2 ** k), Alu.is_gt, Alu.mult
                )
                nc.tensor.matmul(
                    wp[:], lhsT=i128_sb[:], rhs=b_t[:],
                    start=(k == 0), stop=(k == K - 1),
                )
            wt = words_pool.tile([128, FR], i16)
            nc.vector.tensor_copy(wt[:], wp[:])
            wt_tiles.append(wt)
            # conf = prod_k (1 + |h_k|)
            conf_t = conf_pool.tile([128, FR], f16)
            t0 = bb_pool.tile([128, FR], f16)
            nc.vector.scalar_tensor_tensor(
                t0[:], h_t[:, 0, :], -1.0, h_t[:, 0, :], Alu.mult, Alu.max
            )
            nc.vector.tensor_scalar(conf_t[:], t0[:], 1.0, None, Alu.add)
            for k in range(1, K):
                u_t = bb_pool.tile([128, FR], f16)
                nc.vector.scalar_tensor_tensor(
                    u_t[:], h_t[:, k, :], -1.0, h_t[:, k, :], Alu.mult, Alu.max
                )
                nc.vector.scalar_tensor_tensor(
                    conf_t[:], u_t[:], 1.0, conf_t[:], Alu.add, Alu.mult
                )
            conf_tiles.append(conf_t)

        for ii in range(NG):
            i_loc = g * NG + ii
            isl = slice(ii * HP, (ii + 1) * HP)
            # partition-fold: [x=8*p16+xb, y] -> [16m+p16, c=(xb,y)]
            idxw = idxw_pool.tile([128, CI], i16)
            confw = confw_pool.tile([128, CI], f16)
            for m in range(M):
                nc.sync.dma_start(idxw[16 * m : 16 * m + 16, :], wt_tiles[m][:, isl])
                nc.sync.dma_start(
                    confw[16 * m : 16 * m + 16, :], conf_tiles[m][:, isl]
                )
            vsb = vsb_pool.tile([32, QI], f16)
            for ch in range(NCH):
                csl = slice(ch * CCH, (ch + 1) * CCH)
                g_t = g_pool.tile([128, QCH, 2], f16)
                nc.gpsimd.ap_gather(
                    out_ap=g_t[:], in_ap=tblr_sb[:], idxs_ap=idxw[:, csl],
                    channels=128, num_elems=4096, d=2, num_idxs=QCH,
                )
                # conf broadcast [16m+p16] -> [16m+dblk] for each p16
                cbc = cbc_pool.tile([128, P16, CCH], f16)
                for s in range(P16):
                    cp = cp_pool.tile([128, CCH], f32)
                    nc.tensor.matmul(
                        cp[:], lhsT=l16_sb[:, s, :], rhs=confw[:, csl],
                        start=True, stop=True,
                    )
                    nc.vector.tensor_copy(cbc[:, s, :], cp[:])
                # wg = g * conf (slot order (c, p16, e); pair-dup via stride-0)
                gap = g_t[:]
                cap = cbc[:]
                cv = bass.AP(cap.tensor, cap.offset,
                             [cap.ap[0], [1, CCH], [CCH, P16], [0, 2]])
                nc.vector.tensor_tensor(gap, gap, cv, Alu.mult)
                # m-reduce + d-unzip: votes[2*dblk+e, slot] = sum_m wg[(m,dblk), slot, e]
                for sub in range(QCH // SUB):
                    vp = vp_pool.tile([32, SUB], f32)
                    for e in range(2):
                        rv = bass.AP(gap.tensor, gap.offset + sub * SUB * 2 + e,
                                     [gap.ap[0], [2, SUB]])
                        nc.tensor.matmul(
                            vp[:], lhsT=r2_sb[:, e, :], rhs=rv,
                            start=(e == 0), stop=(e == 1),
                        )
                    nc.scalar.activation(
                        vsb[:, ch * QCH + sub * SUB : ch * QCH + (sub + 1) * SUB],
                        vp[:], Act.Copy, bias=0.0, scale=BSCALE,
                    )
            # pooling: slot = xb*1920 + y*16 + p16; pixel (x = 8*p16+xb, y)
            # y-taps into slot-order dyx2 (both sides 16-contiguous)
            dyx2 = dyx_pool.tile([32, NXB, PO, P16], f16)
            vap = vsb[:]
            for t in range(7):
                src = bass.AP(vap.tensor, vap.offset + t * P16,
                              [vap.ap[0], [P16 * HP, NXB], [P16, PO], [1, P16]])
                if t == 0:
                    nc.vector.tensor_copy(dyx2[:], src)
                else:
                    nc.vector.tensor_tensor(dyx2[:], src, dyx2[:], Alu.add)
            # repack to x-affine dyx3[y', x = 8*p16+xb] (reuses the vsb region)
            dyx3 = vsb_pool.tile([32, PO, XP], f16)
            d2 = dyx2[:]
            src = bass.AP(d2.tensor, d2.offset,
                          [d2.ap[0], [P16, PO], [1, P16], [PO * P16, NXB]])
            nc.vector.tensor_copy(dyx3[:], src)
            for sl, (y0, y1) in enumerate(((0, 38), (38, 76), (76, PO))):
                po = po_pool.tile([32, 38, PO], f16)
                for t in range(7):
                    src = dyx3[:, y0:y1, t : t + PO]
                    if t == 0:
                        nc.vector.tensor_copy(po[:], src)
                    else:
                        nc.vector.tensor_tensor(po[:], src, po[:], Alu.add)
                nc.sync.dma_start(out_d[i_loc, :, y0:y1, :], po[:])


def _compile(thresholds, chan_idx, offsets):
    key = (thresholds.tobytes(), chan_idx.tobytes(), offsets.tobytes())
    if _CACHE.get("key") == key:
        return _CACHE["nc"]
    from concourse import bacc
    import concourse.tile as tile

    nc = bacc.Bacc("TRN2", target_bir_lowering=False, debug=False)
    with tile.TileContext(nc) as tc:
        with ExitStack() as ctx:
            _build(ctx, nc, tc, thresholds, chan_idx, offsets)
    nc.compile()
    _CACHE["key"] = key
    _CACHE["nc"] = nc
    return nc


def _install_ntff_hook():
    """Recreate the antenv.axon_hooks NTFF-profile hook this image lacks."""
    import types
    import antenv

    if getattr(antenv, "axon_hooks", None) is not None:
        return
    mod = types.ModuleType("antenv.axon_hooks")
    holder = [None]
    mod.set_axon_ntff_profile_hook = lambda h: holder.__setitem__(0, h)
    mod.get_axon_ntff_profile_hook = lambda: holder[0]
    sys.modules["antenv.axon_hooks"] = mod
    antenv.axon_hooks = mod
    try:
        if "/root/.axon_site" not in sys.path:
            sys.path.insert(0, "/root/.axon_site")
        from trn_agent_boot.trn_boot import _ntff_profile_via_ctypes

        holder[0] = _ntff_profile_via_ctypes("/opt/axon/libaxon_pjrt.so")
    except Exception:
        holder[0] = None


def _run(x, thresholds, table, chan_idx, offsets, trace=False):
    from concourse.bass_utils import run_bass_kernel_spmd

    if trace:
        try:
            _install_ntff_hook()
        except Exception:
            pass

    x = np.asarray(x, np.float32)
    thresholds = np.asarray(thresholds, np.float32)
    table = np.asarray(table, np.float32)
    chan_idx = np.asarray(chan_idx)
    offsets = np.asarray(offsets)

    tblr, ipm, i128, l16, r2 = _host_constants(table)
    thrb = _host_biases(thresholds)
    xt = np.ascontiguousarray(x.transpose(3, 0, 1, 2))  # [W, N, C, H]

    nc = _compile(thresholds, chan_idx, offsets)

    in_maps = []
    for c in range(NCORES):
        in_maps.append({
            "xt": np.ascontiguousarray(xt[:, c * NLOC : (c + 1) * NLOC]),
            "tblr": tblr,
            "ipm": ipm,
            "i128": i128,
            "l16": l16,
            "r2": r2,
            "thrb": thrb,
        })
    res = run_bass_kernel_spmd(nc, in_maps, core_ids=list(range(NCORES)), trace=trace)
    out = np.concatenate(
        [res.results[c]["out"].astype(np.float32).reshape(NLOC, -1)
         for c in range(NCORES)], 0
    )
    return out, res


def kernel(x, thresholds, table, chan_idx, offsets):
    out, _ = _run(x, thresholds, table, chan_idx, offsets)
    return out


# revision 13
# speedup vs baseline: 2.5131x; 1.0052x over previous
"""Trainium2 Bass kernel for the random-fern VQ-codebook problem (nn_CTE_37512244364031).

v3: data-parallel over batch N across 8 NeuronCores (8 images/core), with the
per-pixel table lookup done by gpsimd.ap_gather from an SBUF-resident table
(no per-row SWDGE DMAs, which dominated the old kernel at ~8.7ms/core).

Pixel slot scheme (per image): slot j = c*16 + p16 with c = xb*120 + y,
pixel (x = 8*p16 + xb, y); x in [120,128) (p16 == 15) is junk, never read
by pooling. The wrapped ap_gather index layout [16m+p16, c] is produced from
the stage-A-native word tile [x, (n,y)] by a pure partition-fold DMA
(flat orders match exactly).

Per core:
  stage A (per fern m, per bit k), layout [x=128 partitions, (n4, y120)]:
    - diff via 2 PE matmuls with +/- shifted identities (fp32, exact)
    - ACT tanh(5 z - 5 thr) via per-plane bias column -> h fp16
    - word = sum_k 2^k (h_k > 0): DVE bits + PE identity-matmul accumulation,
      drained to int16 (ap_gather indices)
    - conf = prod_k (1 + |h_k|) via DVE STT chain (2^-12 folded into scales)
  stage B (per image):
    - fold DMAs: words/conf [x, y] -> wrapped [16m+p16, (xb, y)]
    - ap_gather: g[16m+dblk, slot, 2] = table[m, w_m(slot), 2dblk:2dblk+2]
    - conf broadcast to 128 partitions via 16 PE selection matmuls
    - wg = g * conf (DVE, stride-0 pair-dup)
    - m-reduce: 2 e-pass PE matmuls (lhsT R2) -> votes [32 d, slot] PSUM,
      drained fp16 with the 2^-6/49 scale
    - pooling: 7-tap y-adds into x-contiguous layout, then 7-tap x-adds
    - out fp16 [d, y', x'] DMA'd per image; host casts to f32
"""

import os
import sys
from contextlib import ExitStack

import numpy as np

sys.path.insert(0, "/opt/trn_rl_repo")

M, K, L, D = 8, 12, 9, 32
N, C, H, W = 64, 8, 128, 128
HP = WP = 120
PO = 114
NCORES = 8
NLOC = N // NCORES          # 8 images per core
NG = 2                      # images per stage-A group
GROUPS = NLOC // NG
FR = NG * HP                # 480
XP = 128                    # padded x extent
P16 = 16
NXB = 8                     # xb blocks (x = 8*p16 + xb)
CI = NXB * HP               # 960 c-columns per image
QI = CI * P16               # 15360 slots per image
CCH = 120                   # c-columns per gather chunk
QCH = CCH * P16             # 1920 slots per chunk
NCH = CI // CCH             # 8 chunks
SUB = 480                   # slots per m-reduce PSUM chunk
TSCALE = 2.0 ** -6          # folded into table
BSCALE = 2.0 ** -6 / 49.0   # applied at the votes drain

_CACHE = {}


def _host_constants(table):
    # tblr[16m+dblk, w, e] = table[m, w, 2*dblk+e] * TSCALE
    tblr = np.empty((128, 4096, 2), np.float16)
    for m in range(M):
        for dblk in range(16):
            tblr[16 * m + dblk] = (table[m][:, 2 * dblk : 2 * dblk + 2] * TSCALE)
    tblr = np.ascontiguousarray(tblr)

    # shifted +/- identities: ipm[j+s, s, j] = +1, ipm[j+s, 9+s, j] = -1
    ipm = np.zeros((128, 18, XP), np.float32)  # cast to bf16 at the end
    for s in range(9):
        for j in range(XP):
            if j + s < 128:
                ipm[j + s, s, j] = 1.0
                ipm[j + s, 9 + s, j] = -1.0

    i128 = np.eye(128, dtype=np.float16)

    # L16[16m+s, s, 16m+d16] = 1  (conf partition-broadcast selectors)
    l16 = np.zeros((128, P16, 128), np.float16)
    for m in range(M):
        for s in range(P16):
            for d16 in range(P16):
                l16[16 * m + s, s, 16 * m + d16] = 1.0

    # R2[16m+dblk, e, 2dblk+e] = 1  (m-reduce / d-unzip selectors)
    r2 = np.zeros((128, 2, D), np.float16)
    for m in range(M):
        for dblk in range(P16):
            for e in range(2):
                r2[16 * m + dblk, e, 2 * dblk + e] = 1.0

    return tblr, ipm, i128, l16, r2


def _host_biases(thresholds):
    # -5*thr per (m,k) + a zeros column, replicated across partitions
    b = np.zeros((1, M * K + 1), np.float32)
    b[0, : M * K] = -5.0 * thresholds.reshape(M * K)
    return np.ascontiguousarray(np.repeat(b, 128, axis=0))


def _build(ctx, nc, tc, thresholds, chan_idx, offsets):
    import concourse.bass as bass
    from concourse import mybir

    f32 = mybir.dt.float32
    f16 = mybir.dt.float16
    i16 = mybir.dt.int16
    Alu = mybir.AluOpType
    Act = mybir.ActivationFunctionType

    xt_d = nc.dram_tensor("xt", [128, NLOC, C, H], f32, kind="ExternalInput").ap()
    tblr_d = nc.dram_tensor("tblr", [128, 4096, 2], f16, kind="ExternalInput").ap()
    ipm_d = nc.dram_tensor("ipm", [128, 18, XP], f32, kind="ExternalInput").ap()
    i128_d = nc.dram_tensor("i128", [128, 128], f16, kind="ExternalInput").ap()
    l16_d = nc.dram_tensor("l16", [128, P16, 128], f16, kind="ExternalInput").ap()
    r2_d = nc.dram_tensor("r2", [128, 2, D], f16, kind="ExternalInput").ap()
    thrb_d = nc.dram_tensor("thrb", [128, M * K + 1], f32, kind="ExternalInput").ap()
    out_d = nc.dram_tensor("out", [NLOC, D, PO, PO], f16, kind="ExternalOutput").ap()

    const = ctx.enter_context(tc.tile_pool(name="const", bufs=1))
    zp_pool = ctx.enter_context(tc.tile_pool(name="zp", bufs=2, space="PSUM"))
    wp_pool = ctx.enter_context(tc.tile_pool(name="wp", bufs=2, space="PSUM"))
    cp_pool = ctx.enter_context(tc.tile_pool(name="cp", bufs=2, space="PSUM"))
    vp_pool = ctx.enter_context(tc.tile_pool(name="vp", bufs=2, space="PSUM"))
    h_pool = ctx.enter_context(tc.tile_pool(name="h", bufs=1))
    bb_pool = ctx.enter_context(tc.tile_pool(name="bb", bufs=2))
    words_pool = ctx.enter_context(tc.tile_pool(name="words", bufs=12))
    conf_pool = ctx.enter_context(tc.tile_pool(name="confp", bufs=12))
    idxw_pool = ctx.enter_context(tc.tile_pool(name="idxw", bufs=1))
    confw_pool = ctx.enter_context(tc.tile_pool(name="confw", bufs=1))
    g_pool = ctx.enter_context(tc.tile_pool(name="g", bufs=2))
    cbc_pool = ctx.enter_context(tc.tile_pool(name="cbc", bufs=2))
    vsb_pool = ctx.enter_context(tc.tile_pool(name="vsb", bufs=1))
    dyx_pool = ctx.enter_context(tc.tile_pool(name="dyx", bufs=1))
    po_pool = ctx.enter_context(tc.tile_pool(name="po", bufs=1))

    x_sb = const.tile([128, NLOC, C, H], f32)
    nc.sync.dma_start(x_sb[:], xt_d[:])
    tblr_sb = const.tile([128, 4096, 2], f16)
    nc.sync.dma_start(tblr_sb[:], tblr_d[:])
    ipm_sb = const.tile([128, 18, XP], f32)
    nc.sync.dma_start(ipm_sb[:], ipm_d[:])
    i128_sb = const.tile([128, 128], f16)
    nc.sync.dma_start(i128_sb[:], i128_d[:])
    l16_sb = const.tile([128, P16, 128], f16)
    nc.sync.dma_start(l16_sb[:], l16_d[:])
    r2_sb = const.tile([128, 2, D], f16)
    nc.sync.dma_start(r2_sb[:], r2_d[:])
    thrb_sb = const.tile([128, M * K + 1], f32)
    nc.sync.dma_start(thrb_sb[:], thrb_d[:])

    for g in range(GROUPS):
        nsl = slice(g * NG, (g + 1) * NG)
        wt_tiles = []
        conf_tiles = []
        for m in range(M):
            h_t = h_pool.tile([128, K, FR], f16)
            for k in range(K):
                c1, c2 = int(chan_idx[m, k, 0]), int(chan_idx[m, k, 1])
                dy1, dx1 = int(offsets[m, k, 0, 0]), int(offsets[m, k, 0, 1])
                dy2, dx2 = int(offsets[m, k, 1, 0]), int(offsets[m, k, 1, 1])
                mk = m * K + k
                zp = zp_pool.tile([128, FR], f32)
                nc.tensor.matmul(
                    zp[:], lhsT=ipm_sb[:, dx1, :],
                    rhs=x_sb[:, nsl, c1, dy1 : dy1 + HP],
                    start=True, stop=False,
                )
                nc.tensor.matmul(
                    zp[:], lhsT=ipm_sb[:, 9 + dx2, :],
                    rhs=x_sb[:, nsl, c2, dy2 : dy2 + HP],
                    start=False, stop=True,
                )
                nc.scalar.activation(
                    h_t[:, k, :], zp[:], Act.Tanh,
                    bias=thrb_sb[:, mk : mk + 1], scale=5.0,
                )
            # word = sum_k 2^k [h_k > 0] via PE accumulation; drain to int16
            wp = wp_pool.tile([128, FR], f32)
            for k in range(K):
                b_t = bb_pool.tile([128, FR], f16)
                nc.vector.tensor_scalar(
                    b_t[:], h_t[:, k, :], 0.0, float(2 ** k), Alu.is_gt, Alu.mult
                )
                nc.tensor.matmul(
                    wp[:], lhsT=i128_sb[:], rhs=b_t[:],
                    start=(k == 0), stop=(k == K - 1),
                )
            wt = words_pool.tile([128, FR], i16)
            nc.vector.tensor_copy(wt[:], wp[:])
            wt_tiles.append(wt)
            # conf = prod_k (1 + |h_k|)
            conf_t = conf_pool.tile([128, FR], f16)
            t0 = bb_pool.tile([128, FR], f16)
            nc.vector.scalar_tensor_tensor(
                t0[:], h_t[:, 0, :], -1.0, h_t[:, 0, :], Alu.mult, Alu.max
            )
            nc.vector.tensor_scalar(conf_t[:], t0[:], 1.0, None, Alu.add)
            for k in range(1, K):
                u_t = bb_pool.tile([128, FR], f16)
                nc.vector.scalar_tensor_tensor(
                    u_t[:], h_t[:, k, :], -1.0, h_t[:, k, :], Alu.mult, Alu.max
                )
                nc.vector.scalar_tensor_tensor(
                    conf_t[:], u_t[:], 1.0, conf_t[:], Alu.add, Alu.mult
                )
            conf_tiles.append(conf_t)

        for ii in range(NG):
            i_loc = g * NG + ii
            isl = slice(ii * HP, (ii + 1) * HP)
            # partition-fold: [x=8*p16+xb, y] -> [16m+p16, c=(xb,y)]
            idxw = idxw_pool.tile([128, CI], i16)
            confw = confw_pool.tile([128, CI], f16)
            for m in range(M):
                nc.sync.dma_start(idxw[16 * m : 16 * m + 16, :], wt_tiles[m][:, isl])
                nc.sync.dma_start(
                    confw[16 * m : 16 * m + 16, :], conf_tiles[m][:, isl]
                )
            vsb = vsb_pool.tile([32, QI], f16)
            dyx2 = dyx_pool.tile([32, NXB, PO, P16], f16)
            for ch in range(NCH):
                csl = slice(ch * CCH, (ch + 1) * CCH)
                g_t = g_pool.tile([128, QCH, 2], f16)
                nc.gpsimd.ap_gather(
                    out_ap=g_t[:], in_ap=tblr_sb[:], idxs_ap=idxw[:, csl],
                    channels=128, num_elems=4096, d=2, num_idxs=QCH,
                )
                # conf broadcast [16m+p16] -> [16m+dblk] for each p16
                cbc = cbc_pool.tile([128, P16, CCH], f16)
                for s in range(P16):
                    cp = cp_pool.tile([128, CCH], f32)
                    nc.tensor.matmul(
                        cp[:], lhsT=l16_sb[:, s, :], rhs=confw[:, csl],
                        start=True, stop=True,
                    )
                    nc.vector.tensor_copy(cbc[:, s, :], cp[:])
                # wg = g * conf (slot order (c, p16, e); pair-dup via stride-0)
                gap = g_t[:]
                cap = cbc[:]
                cv = bass.AP(cap.tensor, cap.offset,
                             [cap.ap[0], [1, CCH], [CCH, P16], [0, 2]])
                nc.vector.tensor_tensor(gap, gap, cv, Alu.mult)
                # m-reduce + d-unzip: votes[2*dblk+e, slot] = sum_m wg[(m,dblk), slot, e]
                for sub in range(QCH // SUB):
                    vp = vp_pool.tile([32, SUB], f32)
                    for e in range(2):
                        rv = bass.AP(gap.tensor, gap.offset + sub * SUB * 2 + e,
                                     [gap.ap[0], [2, SUB]])
                        nc.tensor.matmul(
                            vp[:], lhsT=r2_sb[:, e, :], rhs=rv,
                            start=(e == 0), stop=(e == 1),
                        )
                    nc.scalar.activation(
                        vsb[:, ch * QCH + sub * SUB : ch * QCH + (sub + 1) * SUB],
                        vp[:], Act.Copy, bias=0.0, scale=BSCALE,
                    )
                # chunk ch covers exactly xb == ch: y-taps chunk-locally so the
                # vsb region frees as soon as the last chunk's taps are done
                vap = vsb[:]
                d2c = dyx2[:, ch]
                for t in range(7):
                    src = bass.AP(vap.tensor, vap.offset + ch * QCH + t * P16,
                                  [vap.ap[0], [P16, PO], [1, P16]])
                    if t == 0:
                        nc.vector.tensor_copy(d2c, src)
                    else:
                        nc.vector.tensor_tensor(d2c, src, d2c, Alu.add)
            # repack to x-affine dyx3[y', x = 8*p16+xb] (reuses the vsb region)
            dyx3 = vsb_pool.tile([32, PO, XP], f16)
            d2 = dyx2[:]
            src = bass.AP(d2.tensor, d2.offset,
                          [d2.ap[0], [P16, PO], [1, P16], [PO * P16, NXB]])
            nc.vector.tensor_copy(dyx3[:], src)
            for sl, (y0, y1) in enumerate(((0, 38), (38, 76), (76, PO))):
                po = po_pool.tile([32, 38, PO], f16)
                for t in range(7):
                    src = dyx3[:, y0:y1, t : t + PO]
                    if t == 0:
                        nc.vector.tensor_copy(po[:], src)
                    else:
                        nc.vector.tensor_tensor(po[:], src, po[:], Alu.add)
                nc.sync.dma_start(out_d[i_loc, :, y0:y1, :], po[:])


def _compile(thresholds, chan_idx, offsets):
    key = (thresholds.tobytes(), chan_idx.tobytes(), offsets.tobytes())
    if _CACHE.get("key") == key:
        return _CACHE["nc"]
    from concourse import bacc
    import concourse.tile as tile

    nc = bacc.Bacc("TRN2", target_bir_lowering=False, debug=False)
    with tile.TileContext(nc) as tc:
        with ExitStack() as ctx:
            _build(ctx, nc, tc, thresholds, chan_idx, offsets)
    nc.compile()
    _CACHE["key"] = key
    _CACHE["nc"] = nc
    return nc


def _install_ntff_hook():
    """Recreate the antenv.axon_hooks NTFF-profile hook this image lacks."""
    import types
    import antenv

    if getattr(antenv, "axon_hooks", None) is not None:
        return
    mod = types.ModuleType("antenv.axon_hooks")
    holder = [None]
    mod.set_axon_ntff_profile_hook = lambda h: holder.__setitem__(0, h)
    mod.get_axon_ntff_profile_hook = lambda: holder[0]
    sys.modules["antenv.axon_hooks"] = mod
    antenv.axon_hooks = mod
    try:
        if "/root/.axon_site" not in sys.path:
            sys.path.insert(0, "/root/.axon_site")
        from trn_agent_boot.trn_boot import _ntff_profile_via_ctypes

        holder[0] = _ntff_profile_via_ctypes("/opt/axon/libaxon_pjrt.so")
    except Exception:
        holder[0] = None


def _run(x, thresholds, table, chan_idx, offsets, trace=False):
    from concourse.bass_utils import run_bass_kernel_spmd

    if trace:
        try:
            _install_ntff_hook()
        except Exception:
            pass

    x = np.asarray(x, np.float32)
    thresholds = np.asarray(thresholds, np.float32)
    table = np.asarray(table, np.float32)
    chan_idx = np.asarray(chan_idx)
    offsets = np.asarray(offsets)

    tblr, ipm, i128, l16, r2 = _host_constants(table)
    thrb = _host_biases(thresholds)
    xt = np.ascontiguousarray(x.transpose(3, 0, 1, 2))  # [W, N, C, H]

    nc = _compile(thresholds, chan_idx, offsets)

    in_maps = []
    for c in range(NCORES):
        in_maps.append({
            "xt": np.ascontiguousarray(xt[:, c * NLOC : (c + 1) * NLOC]),
            "tblr": tblr,
            "ipm": ipm,
            "i128": i128,
            "l16": l16,
            "r2": r2,
            "thrb": thrb,
        })
    res = run_bass_kernel_spmd(nc, in_maps, core_ids=list(range(NCORES)), trace=trace)
    out = np.concatenate(
        [res.results[c]["out"].astype(np.float32).reshape(NLOC, -1)
         for c in range(NCORES)], 0
    )
    return out, res


def kernel(x, thresholds, table, chan_idx, offsets):
    out, _ = _run(x, thresholds, table, chan_idx, offsets)
    return out


# revision 16
# speedup vs baseline: 2.5564x; 1.0172x over previous
"""Trainium2 Bass kernel for the random-fern VQ-codebook problem (nn_CTE_37512244364031).

v3: data-parallel over batch N across 8 NeuronCores (8 images/core), with the
per-pixel table lookup done by gpsimd.ap_gather from an SBUF-resident table
(no per-row SWDGE DMAs, which dominated the old kernel at ~8.7ms/core).

Pixel slot scheme (per image): slot j = c*16 + p16 with c = xb*120 + y,
pixel (x = 8*p16 + xb, y); x in [120,128) (p16 == 15) is junk, never read
by pooling. The wrapped ap_gather index layout [16m+p16, c] is produced from
the stage-A-native word tile [x, (n,y)] by a pure partition-fold DMA
(flat orders match exactly).

Per core:
  stage A (per fern m, per bit k), layout [x=128 partitions, (n4, y120)]:
    - diff via 2 PE matmuls with +/- shifted identities (fp32, exact)
    - ACT tanh(5 z - 5 thr) via per-plane bias column -> h fp16
    - word = sum_k 2^k (h_k > 0): DVE bits + PE identity-matmul accumulation,
      drained to int16 (ap_gather indices)
    - conf = prod_k (1 + |h_k|) via DVE STT chain (2^-12 folded into scales)
  stage B (per image):
    - fold DMAs: words/conf [x, y] -> wrapped [16m+p16, (xb, y)]
    - ap_gather: g[16m+dblk, slot, 2] = table[m, w_m(slot), 2dblk:2dblk+2]
    - conf broadcast to 128 partitions via 16 PE selection matmuls
    - wg = g * conf (DVE, stride-0 pair-dup)
    - m-reduce: 2 e-pass PE matmuls (lhsT R2) -> votes [32 d, slot] PSUM,
      drained fp16 with the 2^-6/49 scale
    - pooling: 7-tap y-adds into x-contiguous layout, then 7-tap x-adds
    - out fp16 [d, y', x'] DMA'd per image; host casts to f32
"""

import os
import sys
from contextlib import ExitStack

import numpy as np

sys.path.insert(0, "/opt/trn_rl_repo")

M, K, L, D = 8, 12, 9, 32
N, C, H, W = 64, 8, 128, 128
HP = WP = 120
PO = 114
NCORES = 8
NLOC = N // NCORES          # 8 images per core
NG = 2                      # images per stage-A group
GROUPS = NLOC // NG
FR = NG * HP                # 480
XP = 128                    # padded x extent
P16 = 16
NXB = 8                     # xb blocks (x = 8*p16 + xb)
CI = NXB * HP               # 960 c-columns per image
QI = CI * P16               # 15360 slots per image
CCH = 120                   # c-columns per gather chunk
QCH = CCH * P16             # 1920 slots per chunk
NCH = CI // CCH             # 8 chunks
SUB = 480                   # slots per m-reduce PSUM chunk
TSCALE = 2.0 ** -6          # folded into table
BSCALE = 2.0 ** -6 / 49.0   # applied at the votes drain

_CACHE = {}


def _host_constants(table):
    # tblr[16m+dblk, w, e] = table[m, w, 2*dblk+e] * TSCALE
    tblr = np.empty((128, 4096, 2), np.float16)
    for m in range(M):
        for dblk in range(16):
            tblr[16 * m + dblk] = (table[m][:, 2 * dblk : 2 * dblk + 2] * TSCALE)
    tblr = np.ascontiguousarray(tblr)

    # shifted +/- identities: ipm[j+s, s, j] = +1, ipm[j+s, 9+s, j] = -1
    ipm = np.zeros((128, 18, XP), np.float32)  # cast to bf16 at the end
    for s in range(9):
        for j in range(XP):
            if j + s < 128:
                ipm[j + s, s, j] = 1.0
                ipm[j + s, 9 + s, j] = -1.0

    i128 = np.eye(128, dtype=np.float16)

    # L16[16m+s, s, 16m+d16] = 1  (conf partition-broadcast selectors)
    l16 = np.zeros((128, P16, 128), np.float16)
    for m in range(M):
        for s in range(P16):
            for d16 in range(P16):
                l16[16 * m + s, s, 16 * m + d16] = 1.0

    # R2[16m+dblk, e, 2dblk+e] = 1  (m-reduce / d-unzip selectors)
    r2 = np.zeros((128, 2, D), np.float16)
    for m in range(M):
        for dblk in range(P16):
            for e in range(2):
                r2[16 * m + dblk, e, 2 * dblk + e] = 1.0

    return tblr, ipm, i128, l16, r2


def _host_biases(thresholds):
    # -5*thr per (m,k) + a zeros column, replicated across partitions
    b = np.zeros((1, M * K + 1), np.float32)
    b[0, : M * K] = -5.0 * thresholds.reshape(M * K)
    return np.ascontiguousarray(np.repeat(b, 128, axis=0))


def _build(ctx, nc, tc, thresholds, chan_idx, offsets):
    import concourse.bass as bass
    from concourse import mybir

    f32 = mybir.dt.float32
    f16 = mybir.dt.float16
    i16 = mybir.dt.int16
    Alu = mybir.AluOpType
    Act = mybir.ActivationFunctionType

    xt_d = nc.dram_tensor("xt", [128, NLOC, C, H], f32, kind="ExternalInput").ap()
    tblr_d = nc.dram_tensor("tblr", [128, 4096, 2], f16, kind="ExternalInput").ap()
    ipm_d = nc.dram_tensor("ipm", [128, 18, XP], f32, kind="ExternalInput").ap()
    i128_d = nc.dram_tensor("i128", [128, 128], f16, kind="ExternalInput").ap()
    l16_d = nc.dram_tensor("l16", [128, P16, 128], f16, kind="ExternalInput").ap()
    r2_d = nc.dram_tensor("r2", [128, 2, D], f16, kind="ExternalInput").ap()
    thrb_d = nc.dram_tensor("thrb", [128, M * K + 1], f32, kind="ExternalInput").ap()
    out_d = nc.dram_tensor("out", [NLOC, D, PO, PO], f16, kind="ExternalOutput").ap()

    const = ctx.enter_context(tc.tile_pool(name="const", bufs=1))
    zp_pool = ctx.enter_context(tc.tile_pool(name="zp", bufs=2, space="PSUM"))
    wp_pool = ctx.enter_context(tc.tile_pool(name="wp", bufs=2, space="PSUM"))
    cp_pool = ctx.enter_context(tc.tile_pool(name="cp", bufs=2, space="PSUM"))
    vp_pool = ctx.enter_context(tc.tile_pool(name="vp", bufs=2, space="PSUM"))
    h_pool = ctx.enter_context(tc.tile_pool(name="h", bufs=1))
    bb_pool = ctx.enter_context(tc.tile_pool(name="bb", bufs=2))
    words_pool = ctx.enter_context(tc.tile_pool(name="words", bufs=12))
    conf_pool = ctx.enter_context(tc.tile_pool(name="confp", bufs=12))
    idxw_pool = ctx.enter_context(tc.tile_pool(name="idxw", bufs=2))
    confw_pool = ctx.enter_context(tc.tile_pool(name="confw", bufs=2))
    g_pool = ctx.enter_context(tc.tile_pool(name="g", bufs=2))
    cbc_pool = ctx.enter_context(tc.tile_pool(name="cbc", bufs=2))
    vsb_pool = ctx.enter_context(tc.tile_pool(name="vsb", bufs=1))
    dyx_pool = ctx.enter_context(tc.tile_pool(name="dyx", bufs=1))
    po_pool = ctx.enter_context(tc.tile_pool(name="po", bufs=1))

    x_sb = const.tile([128, NLOC, C, H], f32)
    nc.sync.dma_start(x_sb[:], xt_d[:])
    tblr_sb = const.tile([128, 4096, 2], f16)
    nc.sync.dma_start(tblr_sb[:], tblr_d[:])
    ipm_sb = const.tile([128, 18, XP], f32)
    nc.sync.dma_start(ipm_sb[:], ipm_d[:])
    i128_sb = const.tile([128, 128], f16)
    nc.sync.dma_start(i128_sb[:], i128_d[:])
    l16_sb = const.tile([128, P16, 128], f16)
    nc.sync.dma_start(l16_sb[:], l16_d[:])
    r2_sb = const.tile([128, 2, D], f16)
    nc.sync.dma_start(r2_sb[:], r2_d[:])
    thrb_sb = const.tile([128, M * K + 1], f32)
    nc.sync.dma_start(thrb_sb[:], thrb_d[:])

    def emit_stage_a(g, m):
        nsl = slice(g * NG, (g + 1) * NG)
        h_t = h_pool.tile([128, K, FR], f16)
        for k in range(K):
            c1, c2 = int(chan_idx[m, k, 0]), int(chan_idx[m, k, 1])
            dy1, dx1 = int(offsets[m, k, 0, 0]), int(offsets[m, k, 0, 1])
            dy2, dx2 = int(offsets[m, k, 1, 0]), int(offsets[m, k, 1, 1])
            mk = m * K + k
            zp = zp_pool.tile([128, FR], f32)
            nc.tensor.matmul(
                zp[:], lhsT=ipm_sb[:, dx1, :],
                rhs=x_sb[:, nsl, c1, dy1 : dy1 + HP],
                start=True, stop=False,
            )
            nc.tensor.matmul(
                zp[:], lhsT=ipm_sb[:, 9 + dx2, :],
                rhs=x_sb[:, nsl, c2, dy2 : dy2 + HP],
                start=False, stop=True,
            )
            nc.scalar.activation(
                h_t[:, k, :], zp[:], Act.Tanh,
                bias=thrb_sb[:, mk : mk + 1], scale=5.0,
            )
        # word = sum_k 2^k [h_k > 0] via PE accumulation; drain to int16
        wp = wp_pool.tile([128, FR], f32)
        for k in range(K):
            b_t = bb_pool.tile([128, FR], f16)
            nc.vector.tensor_scalar(
                b_t[:], h_t[:, k, :], 0.0, float(2 ** k), Alu.is_gt, Alu.mult
            )
            nc.tensor.matmul(
                wp[:], lhsT=i128_sb[:], rhs=b_t[:],
                start=(k == 0), stop=(k == K - 1),
            )
        wt = words_pool.tile([128, FR], i16)
        nc.vector.tensor_copy(wt[:], wp[:])
        # conf = prod_k (1 + |h_k|)
        conf_t = conf_pool.tile([128, FR], f16)
        t0 = bb_pool.tile([128, FR], f16)
        nc.vector.scalar_tensor_tensor(
            t0[:], h_t[:, 0, :], -1.0, h_t[:, 0, :], Alu.mult, Alu.max
        )
        nc.vector.tensor_scalar(conf_t[:], t0[:], 1.0, None, Alu.add)
        for k in range(1, K):
            u_t = bb_pool.tile([128, FR], f16)
            nc.vector.scalar_tensor_tensor(
                u_t[:], h_t[:, k, :], -1.0, h_t[:, k, :], Alu.mult, Alu.max
            )
            nc.vector.scalar_tensor_tensor(
                conf_t[:], u_t[:], 1.0, conf_t[:], Alu.add, Alu.mult
            )
        return wt, conf_t

    def emit_folds(tiles_g, ii):
        isl = slice(ii * HP, (ii + 1) * HP)
        idxw = idxw_pool.tile([128, CI], i16)
        confw = confw_pool.tile([128, CI], f16)
        for m in range(M):
            wt, conf_t = tiles_g[m]
            nc.sync.dma_start(idxw[16 * m : 16 * m + 16, :], wt[:, isl])
            nc.sync.dma_start(confw[16 * m : 16 * m + 16, :], conf_t[:, isl])
        return idxw, confw

    tiles = {0: [emit_stage_a(0, m) for m in range(M)]}
    for g in range(GROUPS):
        folded = [emit_folds(tiles[g], ii) for ii in range(NG)]
        for ii in range(NG):
            i_loc = g * NG + ii
            idxw, confw = folded[ii]
            vsb = vsb_pool.tile([32, QI], f16)
            dyx2 = dyx_pool.tile([32, NXB, PO, P16], f16)
            for ch in range(NCH):
                csl = slice(ch * CCH, (ch + 1) * CCH)
                g_t = g_pool.tile([128, QCH, 2], f16)
                nc.gpsimd.ap_gather(
                    out_ap=g_t[:], in_ap=tblr_sb[:], idxs_ap=idxw[:, csl],
                    channels=128, num_elems=4096, d=2, num_idxs=QCH,
                )
                # conf broadcast [16m+p16] -> [16m+dblk] for each p16
                cbc = cbc_pool.tile([128, P16, CCH], f16)
                for s_ in range(P16):
                    cp = cp_pool.tile([128, CCH], f32)
                    nc.tensor.matmul(
                        cp[:], lhsT=l16_sb[:, s_, :], rhs=confw[:, csl],
                        start=True, stop=True,
                    )
                    nc.vector.tensor_copy(cbc[:, s_, :], cp[:])
                # wg = g * conf (slot order (c, p16, e); pair-dup via stride-0)
                gap = g_t[:]
                cap = cbc[:]
                cv = bass.AP(cap.tensor, cap.offset,
                             [cap.ap[0], [1, CCH], [CCH, P16], [0, 2]])
                nc.vector.tensor_tensor(gap, gap, cv, Alu.mult)
                # m-reduce + d-unzip: votes[2*dblk+e, slot] = sum_m wg[(m,dblk), slot, e]
                for sub in range(QCH // SUB):
                    vp = vp_pool.tile([32, SUB], f32)
                    for e in range(2):
                        rv = bass.AP(gap.tensor, gap.offset + sub * SUB * 2 + e,
                                     [gap.ap[0], [2, SUB]])
                        nc.tensor.matmul(
                            vp[:], lhsT=r2_sb[:, e, :], rhs=rv,
                            start=(e == 0), stop=(e == 1),
                        )
                    nc.scalar.activation(
                        vsb[:, ch * QCH + sub * SUB : ch * QCH + (sub + 1) * SUB],
                        vp[:], Act.Copy, bias=0.0, scale=BSCALE,
                    )
                # chunk ch covers exactly xb == ch: y-taps chunk-locally so the
                # vsb region frees as soon as the last chunk's taps are done
                vap = vsb[:]
                d2c = dyx2[:, ch]
                for t in range(7):
                    src = bass.AP(vap.tensor, vap.offset + ch * QCH + t * P16,
                                  [vap.ap[0], [P16, PO], [1, P16]])
                    if t == 0:
                        nc.vector.tensor_copy(d2c, src)
                    else:
                        nc.vector.tensor_tensor(d2c, src, d2c, Alu.add)
                # interleave next group's stage A (4 ferns per image's chunks)
                if g + 1 < GROUPS:
                    nxt = tiles.setdefault(g + 1, [])
                    if ii == 0 and 4 <= ch < 8:
                        nxt.append(emit_stage_a(g + 1, ch - 4))
                    elif ii == 1 and ch < 4:
                        nxt.append(emit_stage_a(g + 1, ch + 4))
            # repack to x-affine dyx3[y', x = 8*p16+xb] (reuses the vsb region)
            dyx3 = vsb_pool.tile([32, PO, XP], f16)
            d2 = dyx2[:]
            src = bass.AP(d2.tensor, d2.offset,
                          [d2.ap[0], [P16, PO], [1, P16], [PO * P16, NXB]])
            nc.vector.tensor_copy(dyx3[:], src)
            for sl in range(6):
                y0, y1 = 19 * sl, 19 * (sl + 1)
                po = po_pool.tile([32, 19, PO], f16)
                for t in range(7):
                    src = dyx3[:, y0:y1, t : t + PO]
                    if t == 0:
                        nc.vector.tensor_copy(po[:], src)
                    else:
                        nc.vector.tensor_tensor(po[:], src, po[:], Alu.add)
                nc.sync.dma_start(out_d[i_loc, :, y0:y1, :], po[:])


def _compile(thresholds, chan_idx, offsets):
    key = (thresholds.tobytes(), chan_idx.tobytes(), offsets.tobytes())
    if _CACHE.get("key") == key:
        return _CACHE["nc"]
    from concourse import bacc
    import concourse.tile as tile

    nc = bacc.Bacc("TRN2", target_bir_lowering=False, debug=False)
    with tile.TileContext(nc) as tc:
        with ExitStack() as ctx:
            _build(ctx, nc, tc, thresholds, chan_idx, offsets)
    nc.compile()
    _CACHE["key"] = key
    _CACHE["nc"] = nc
    return nc


def _install_ntff_hook():
    """Recreate the antenv.axon_hooks NTFF-profile hook this image lacks."""
    import types
    import antenv

    if getattr(antenv, "axon_hooks", None) is not None:
        return
    mod = types.ModuleType("antenv.axon_hooks")
    holder = [None]
    mod.set_axon_ntff_profile_hook = lambda h: holder.__setitem__(0, h)
    mod.get_axon_ntff_profile_hook = lambda: holder[0]
    sys.modules["antenv.axon_hooks"] = mod
    antenv.axon_hooks = mod
    try:
        if "/root/.axon_site" not in sys.path:
            sys.path.insert(0, "/root/.axon_site")
        from trn_agent_boot.trn_boot import _ntff_profile_via_ctypes

        holder[0] = _ntff_profile_via_ctypes("/opt/axon/libaxon_pjrt.so")
    except Exception:
        holder[0] = None


def _run(x, thresholds, table, chan_idx, offsets, trace=False):
    from concourse.bass_utils import run_bass_kernel_spmd

    if trace:
        try:
            _install_ntff_hook()
        except Exception:
            pass

    x = np.asarray(x, np.float32)
    thresholds = np.asarray(thresholds, np.float32)
    table = np.asarray(table, np.float32)
    chan_idx = np.asarray(chan_idx)
    offsets = np.asarray(offsets)

    tblr, ipm, i128, l16, r2 = _host_constants(table)
    thrb = _host_biases(thresholds)
    xt = np.ascontiguousarray(x.transpose(3, 0, 1, 2))  # [W, N, C, H]

    nc = _compile(thresholds, chan_idx, offsets)

    in_maps = []
    for c in range(NCORES):
        in_maps.append({
            "xt": np.ascontiguousarray(xt[:, c * NLOC : (c + 1) * NLOC]),
            "tblr": tblr,
            "ipm": ipm,
            "i128": i128,
            "l16": l16,
            "r2": r2,
            "thrb": thrb,
        })
    res = run_bass_kernel_spmd(nc, in_maps, core_ids=list(range(NCORES)), trace=trace)
    out = np.concatenate(
        [res.results[c]["out"].astype(np.float32).reshape(NLOC, -1)
         for c in range(NCORES)], 0
    )
    return out, res


def kernel(x, thresholds, table, chan_idx, offsets):
    out, _ = _run(x, thresholds, table, chan_idx, offsets)
    return out
